# revision 38
# baseline (speedup 1.0000x reference)
"""DCNv2 (modulated deformable conv k=3 s=1 p=1) + BatchNorm(train) + ReLU on 8 TRN2 cores.

Sharding: data-parallel over batch (1 sample per core); BN statistics all-reduced.

Per-core pipeline (v2 — rebalanced across engines/queues):
  - offset conv (PE, fp32r full-rate) in the slot-permuted column order sigma:
    slot j <-> hw = (j%16)*256 + j//16, so ap_gather's 16-partition index
    wrapping needs no on-chip transpose and out_ps columns come out hw-linear.
  - om PSUM -> [36,1024] quarter-packed maps via SBUF->SBUF partition-strided
    DMAs (no DRAM bounce).
  - per-position index/coefficient math on shared [100,1024] ops (y rows 0:36,
    x rows 64:100); floor via ALU mod; fused closed-form i16 pair indices.
  - bilinear gather as f32 words (= bf16 horizontal pairs, d=1) via GPSIMD
    ap_gather from the parity-duplicated pair image; result bitcast to bf16.
  - per-corner coefficients broadcast to 128 partitions via DRAM bounce, with
    the 2MB broadcast loads alternating between the SP and ACT HWDGE queues.
  - corner products (DVE, bf16 2x); 4-way bilinear sum rides PE PSUM
    accumulation (stride-2 moving operands); 18 half-tap units, triple-buffered.
  - BN stats via bn_stats/bn_aggr (DVE), all-reduced across 8 cores,
    fused scale/bias + ReLU on ACT.
"""

import numpy as np
import ml_dtypes
from contextlib import ExitStack

import bass_rust
import concourse.bass as bass
import concourse.tile as tile
from concourse import bacc, mybir
from concourse.bass_utils import run_bass_kernel_spmd

F32 = mybir.dt.float32
F32R = mybir.dt.float32r
BF16 = mybir.dt.bfloat16
I32 = mybir.dt.int32
I16 = mybir.dt.int16
AF = mybir.ActivationFunctionType
ALU = mybir.AluOpType

B, CHI, CHO, H, W = 8, 128, 128, 64, 64
KK = 9
HW = H * W  # 4096
PADW = 66
NPAD = PADW * PADW  # 4356
EPS = 1e-5


def _ap(base, off, dims):
    """Custom AP rooted at an existing AP `base` (keeps symbolic tile tensor)."""
    return bass_rust.AP(base.tensor, base.offset + off, [list(d) for d in dims])


def build_kernel(n_cores=8):
    nc = bacc.Bacc("TRN2", target_bir_lowering=False, debug=False,
                   num_devices=n_cores)

    x_d = nc.dram_tensor("x", [CHI, HW], F32, kind="ExternalInput")
    offw_d = nc.dram_tensor("offw", [KK, CHI, 27], BF16, kind="ExternalInput")
    w_d = nc.dram_tensor("w", [KK, CHI, CHO], BF16, kind="ExternalInput")
    gridy_d = nc.dram_tensor("gridy", [100, 1024], F32, kind="ExternalInput")
    offbm_d = nc.dram_tensor("offbm", [36, 1], F32, kind="ExternalInput")
    gamma_d = nc.dram_tensor("gamma", [CHO], F32, kind="ExternalInput")
    beta_d = nc.dram_tensor("beta", [CHO], F32, kind="ExternalInput")
    out_d = nc.dram_tensor("out", [CHO, HW], F32, kind="ExternalOutput")

    with tile.TileContext(nc) as tc:
        with ExitStack() as ctx:
            _body(ctx, tc, nc, n_cores,
                  x_d, offw_d, w_d, gridy_d, offbm_d, gamma_d, beta_d,
                  out_d)
    nc.compile()
    return nc


def _body(ctx, tc, nc, n_cores,
          x_d, offw_d, w_d, gridy_d, offbm_d, gamma_d, beta_d, out_d):
    consts = ctx.enter_context(tc.tile_pool(name="consts", bufs=1))
    xpool = ctx.enter_context(tc.tile_pool(name="xpool", bufs=1))
    dram = ctx.enter_context(tc.tile_pool(name="dram", bufs=1, space="DRAM"))

    # two HWDGE queues; alternate big transfers between them
    _q = [0]

    def dq():
        _q[0] ^= 1
        return nc.sync if _q[0] else nc.scalar

    # ---- constant loads -------------------------------------------------
    # all consts ride the sync queue: the ACT queue must stay free for the
    # xpad interior copy that gates the offset conv
    offw_sb = consts.tile([CHI, KK * 27], BF16)    # per tap t: cols 27t..27t+27
    nc.sync.dma_start(offw_sb[:],
                      _ap(offw_d.ap(), 0, [[27, CHI], [CHI * 27, KK], [1, 27]]))
    gridy = consts.tile([100, 1024], F32)
    nc.sync.dma_start(gridy[:], gridy_d.ap())
    w_sb = consts.tile([CHI, KK * CHO], BF16)
    nc.sync.dma_start(w_sb[:],
                      _ap(w_d.ap(), 0, [[CHO, CHI], [CHI * CHO, KK], [1, CHO]]))
    offbm = consts.tile([36, 1], F32)
    nc.sync.dma_start(offbm[:], offbm_d.ap())
    gam = consts.tile([CHO, 1], F32)
    nc.sync.dma_start(gam[:], _ap(gamma_d.ap(), 0, [[1, CHO], [1, 1]]))
    bet = consts.tile([CHO, 1], F32)
    nc.sync.dma_start(bet[:], _ap(beta_d.ap(), 0, [[1, CHO], [1, 1]]))

    # ---- scoped: pad image, offset conv, per-position maps --------------
    idramT = dram.tile([KK, HW], I16)
    idramB = dram.tile([KK, HW], I16)
    cdramT = dram.tile([36, 2048], BF16)   # rows 4k+q
    cdramB = dram.tile([36, 2048], BF16)
    xbf = xpool.tile([CHI, 2 * HW], BF16)   # [0:4096]=x, [4096:8191]=x[1:], 0-pad
    with tc.tile_pool(name="maps", bufs=1) as maps, \
         tc.tile_pool(name="pads", bufs=1) as pads:
        xpad = pads.tile([CHI, NPAD], BF16)
        oyx = maps.tile([100, 1024], F32, tag="oyx")
        mk = maps.tile([36, 1024], F32, tag="mk")
        xps = xpad[:].ap[0][0]

        with tc.tile_pool(name="xin", bufs=1) as xin:
            # x halves on both queues: x gates xpad -> conv
            x_sb = xin.tile([CHI, HW], F32)
            nc.sync.dma_start(x_sb[0:64, :],
                              _ap(x_d.ap(), 0, [[HW, 64], [1, HW]]))
            nc.scalar.dma_start(x_sb[64:128, :],
                                _ap(x_d.ap(), 64 * HW, [[HW, 64], [1, HW]]))
            # zero only the 1-pixel pad border; interior is overwritten
            nc.vector.memset(_ap(xpad[:], 0, [[xps, CHI], [1, PADW]]), 0.0)
            nc.vector.memset(
                _ap(xpad[:], 65 * PADW, [[xps, CHI], [1, PADW]]), 0.0)
            nc.vector.memset(
                _ap(xpad[:], PADW, [[xps, CHI], [PADW, 64], [1, 1]]), 0.0)
            nc.vector.memset(
                _ap(xpad[:], PADW + 65, [[xps, CHI], [PADW, 64], [1, 1]]), 0.0)
            # interior copy on ACT: pad[(y+1)*66 + (x+1)] = x[y*64 + x]
            xss = x_sb[:].ap[0][0]
            nc.scalar.activation(
                _ap(xpad[:], PADW + 1, [[xps, CHI], [PADW, H], [1, W]]),
                _ap(x_sb[:], 0, [[xss, CHI], [W, H], [1, W]]), AF.Copy)
            # bf16 pair image straight from DRAM (gpsimd DMAs may cast)
            nc.vector.memset(xbf[:, 2 * HW - 1:2 * HW], 0.0)
            nc.gpsimd.dma_start(xbf[:, 0:HW], x_d.ap())
            nc.gpsimd.dma_start(xbf[:, HW:2 * HW - 1],
                                _ap(x_d.ap(), 1, [[HW, CHI], [1, HW - 1]]))

        # ---- offset conv (slot-ordered columns, bf16) -----------------------
        # column c in [0,1024) of quarter q: y = 4*(c//64)+q, x = c%64
        # zero the unused gap rows from gridy's zero rows (DMA: no partition
        # alignment restriction); all oyx writes ride the sync queue so the
        # race detector sees them ordered.
        nc.sync.dma_start(
            oyx[36:64, :],
            _ap(gridy_d.ap(), 36 * 1024, [[1024, 28], [1, 1024]]))
        with tc.tile_pool(name="ompsum", bufs=2, space="PSUM") as omp:
            for q in range(4):
                om_ps = omp.tile([27, 1024], F32, tag="om")
                for t in range(KK):
                    di, dj = t // 3, t % 3
                    for h2 in range(2):
                        rhs = _ap(xpad[:],
                                  (q + di) * PADW + dj + 8 * h2 * 4 * PADW,
                                  [[xps, CHI], [4 * PADW, 8], [1, 64]])
                        nc.tensor.matmul(
                            om_ps[:, 512 * h2:512 * h2 + 512],
                            offw_sb[:, 27 * t:27 * t + 27], rhs,
                            start=(t == 0), stop=(t == KK - 1))
                om_sb = maps.tile([27, 1024], F32, tag=f"om_sb{q}",
                                  name=f"om_sb{q}")
                nc.scalar.activation(om_sb[:], om_ps[:], AF.Copy)
                # map row r = 9q+k: shuffles are contiguous tile slices
                nc.sync.dma_start(oyx[9 * q:9 * q + 9, :], om_sb[0:9, :])
                nc.sync.dma_start(oyx[64 + 9 * q:64 + 9 * q + 9, :],
                                  om_sb[9:18, :])
                nc.scalar.dma_start(mk[9 * q:9 * q + 9, :], om_sb[18:27, :])

        # ---- per-position math on [100,1024] maps (manual slot reuse) -------
        ts_ = nc.vector.tensor_scalar
        tt = nc.vector.tensor_tensor
        stt = nc.vector.scalar_tensor_tensor

        def T(tag, dt=F32):
            return maps.tile([36, 1024], dt, tag=tag, name=tag)

        def T2(tag, dt=F32):
            return maps.tile([100, 1024], dt, tag=tag, name=tag)

        cp = nc.vector.tensor_copy
        pyx = oyx                              # in-place add
        tt(pyx[:], oyx[:], gridy[:], ALU.add)
        # floor() robust to the convert rounding mode (HW: RNE, sim: trunc)
        ti = T2("u1", I32)
        cp(ti[:], pyx[:])
        fyx = T2("u2")
        cp(fyx[:], ti[:])
        gg = T2("u1b")
        tt(gg[:], fyx[:], pyx[:], ALU.is_gt)
        tt(fyx[:], fyx[:], gg[:], ALU.subtract)
        md = T2("u3"); tt(md[:], pyx[:], fyx[:], ALU.subtract)  # ly | lx
        myx = T2("u4"); ts_(myx[:], md[:], -1.0, 1.0, ALU.mult, ALU.add)
        sig = mk
        nc.scalar.activation(sig[:], mk[:], AF.Sigmoid, bias=offbm[:])
        # in-range indicators (same bounds for y and x halves)
        ca = T2("u1c"); ts_(ca[:], fyx[:], 0.0, 63.0, ALU.max, ALU.min)
        vtl = T2("u5"); tt(vtl[:], ca[:], fyx[:], ALU.is_equal)
        cb2 = T2("u1c2"); ts_(cb2[:], fyx[:], -1.0, 62.0, ALU.max, ALU.min)
        vbr = T2("u6"); tt(vbr[:], cb2[:], fyx[:], ALU.is_equal)
        # wTL/wxL halves and wyB/wxR halves in shared ops
        wA = T2("u7"); tt(wA[:], myx[:], vtl[:], ALU.mult)   # y:(1-ly)vt | x:(1-lx)vl
        wB = T2("u8"); tt(wB[:], md[:], vbr[:], ALU.mult)    # y: ly*vb   | x: lx*vr
        # x0 == -1 pair-base swap, applied to the x halves in place
        slx = T2("u9")
        ts_(slx[64:100, :], fyx[64:100, :], -1.0, None, ALU.is_equal)
        tt(slx[64:100, :], wB[64:100, :], slx[64:100, :], ALU.mult)  # wxR*[fx==-1]
        tt(wA[64:100, :], wA[64:100, :], slx[64:100, :], ALU.add)
        tt(wB[64:100, :], wB[64:100, :], slx[64:100, :], ALU.subtract)
        # mask fold into the y halves
        tt(wA[0:36, :], wA[0:36, :], sig[:], ALU.mult)
        tt(wB[0:36, :], wB[0:36, :], sig[:], ALU.mult)
        # bring x halves onto partitions 0:36 (cross-partition -> DMA)
        wxL = T("t8"); dq().dma_start(wxL[:], wA[64:100, :])
        wxR = T("t9"); dq().dma_start(wxR[:], wB[64:100, :])
        xc = T("t3"); dq().dma_start(xc[:], ca[64:100, :])   # clip(fx,0,63)
        # coef pairs in gather-stream order: map col c=64b+a lands at pair
        # column 2*(16a+b)+e (the 16-partition wrap rides the output AP)
        ctop = maps.tile([36, 2 * 1024], BF16, tag="ctop", name="ctop")
        cbot = maps.tile([36, 2 * 1024], BF16, tag="cbot", name="cbot")
        cts = ctop[:].ap[0][0]
        cbs = cbot[:].ap[0][0]

        def cdims(cs):
            return [[cs, 36], [2, 16], [32, 64]]

        def mdims(m):
            s = m[:].ap[0][0]
            return _ap(m[:], 0, [[s, 36], [64, 16], [1, 64]])

        wAy = _ap(wA[:], 0, [[wA[:].ap[0][0], 36], [64, 16], [1, 64]])
        wBy = _ap(wB[:], 0, [[wB[:].ap[0][0], 36], [64, 16], [1, 64]])
        tt(_ap(ctop[:], 0, cdims(cts)), wAy, mdims(wxL), ALU.mult)
        tt(_ap(ctop[:], 1, cdims(cts)), wAy, mdims(wxR), ALU.mult)
        tt(_ap(cbot[:], 0, cdims(cbs)), wBy, mdims(wxL), ALU.mult)
        tt(_ap(cbot[:], 1, cdims(cbs)), wBy, mdims(wxR), ALU.mult)

        # pair index: ii = par*2048 + (yc*64+xc)>>1  (yc, ycb from the clips)
        ycb = T("t6"); ts_(ycb[:], cb2[0:36, :], 1.0, None, ALU.add)
        pT = T("t2"); stt(pT[:], ca[0:36, :], float(W), xc[:], ALU.mult, ALU.add)
        pB = T("t4b"); stt(pB[:], ycb[:], float(W), xc[:], ALU.mult, ALU.add)
        idx16 = {}
        for name, p in (("T", pT), ("B", pB)):
            pi = T("t5b" + name, I32); cp(pi[:], p[:])
            par = T("t7" + name, I32); ts_(par[:], pi[:], 1, None, ALU.bitwise_and)
            hf = T("t7b" + name, I32)
            ts_(hf[:], pi[:], 1, None, ALU.arith_shift_right)
            ii = T("i" + name, I16)   # i16 convert fused into the combine
            stt(ii[:], par[:], float(HW // 2), hf[:], ALU.mult, ALU.add)
            idx16[name] = ii
        iiT, iiB = idx16["T"], idx16["B"]

        # ---- bounce coef/idx maps to DRAM -----------------------------------
        # cdram rows stay r = 9q+k (linear write; the broadcast read strides q)
        nc.sync.dma_start(
            _ap(cdramT[:], 0, [[2048, 36], [1, 2048]]),
            _ap(ctop[:], 0, [[cts, 36], [1, 2048]]))
        nc.scalar.dma_start(
            _ap(cdramB[:], 0, [[2048, 36], [1, 2048]]),
            _ap(cbot[:], 0, [[cbs, 36], [1, 2048]]))
        # idram[k] entry p' = 256*b + 64*q + a <- ii[9q+k, 64b+a]; per-q DMAs
        # keep both dst and src 3-dim (a permuted 4-dim dst costs ~10x)
        for eng, idram, ii in ((nc.sync, idramT, iiT),
                               (nc.scalar, idramB, iiB)):
            s = ii[:].ap[0][0]
            for q in range(4):
                eng.dma_start(
                    _ap(idram[:], 64 * q, [[HW, 9], [256, 16], [1, 64]]),
                    _ap(ii[:], 9 * q * s, [[s, 9], [64, 16], [1, 64]]))

    # ---- gather + interp + main conv (half-tap pipeline units) ----------
    gpool = ctx.enter_context(tc.tile_pool(name="gpool", bufs=2))
    out_pp = ctx.enter_context(tc.tile_pool(name="outp", bufs=1, space="PSUM"))
    out_ps = out_pp.tile([CHO, HW], F32)
    tt = nc.vector.tensor_tensor
    ts_ = nc.vector.tensor_scalar

    for k in range(KK):
        for hh in range(2):
            u = 2 * k + hh
            qa, qb = (nc.sync, nc.scalar) if u % 2 else (nc.scalar, nc.sync)
            idram = idramT if hh == 0 else idramB
            cdrm = cdramT if hh == 0 else cdramB
            ix = gpool.tile([128, 256], I16, tag="ix", name="ix", bufs=3)
            qa.dma_start(
                ix[:],
                _ap(idram[:], k * HW, [[0, 8], [256, 16], [1, 256]]))
            g = gpool.tile([128, 2 * HW], BF16, tag="g", name="g", bufs=2)
            nc.gpsimd.ap_gather(g[:].bitcast(F32), xbf[:].bitcast(F32), ix[:],
                                channels=128, num_elems=HW, d=1, num_idxs=HW)
            cbt = gpool.tile([128, 2 * HW], BF16, tag="cb", name="cb", bufs=3)
            qb.dma_start(
                cbt[:],
                _ap(cdrm[:], k * 2048, [[0, 128], [9 * 2048, 4], [1, 2048]]))
            p = gpool.tile([128, 2 * HW], BF16, tag="prod", name="prod", bufs=2)
            tt(p[:], cbt[:], g[:], ALU.mult)
            ps_ = p[:].ap[0][0]
            for par in (0, 1):
                for c8 in range(8):
                    rhs = _ap(p[:], 4 * c8 + par,
                              [[ps_, 128], [2, 2], [32, 256]])
                    nc.tensor.matmul(
                        out_ps[:, 512 * c8:512 * c8 + 512],
                        w_sb[:, CHO * k:CHO * k + CHO],
                        rhs, start=(k == 0 and hh == 0 and par == 0),
                        stop=(k == KK - 1 and hh == 1 and par == 1))

    # ---- BatchNorm (bn_stats + all-reduce) + ReLU -----------------------
    bn = ctx.enter_context(tc.tile_pool(name="bn", bufs=1))
    bns = bn.tile([CHO, 48], F32)
    for c8 in range(8):
        nc.vector.bn_stats(bns[:, 6 * c8:6 * c8 + 6],
                           out_ps[:, 512 * c8:512 * c8 + 512])
    mv = bn.tile([CHO, 2], F32)
    nc.vector.bn_aggr(mv[:], bns[:])
    # per-core sums: s1 = mean*HW, s2 = (var + mean^2)*HW
    ccs = bn.tile([CHO, 2], F32)
    ts_(ccs[:, 0:1], mv[:, 0:1], float(HW), None, ALU.mult)
    msq = bn.tile([CHO, 1], F32)
    tt(msq[:], mv[:, 0:1], mv[:, 0:1], ALU.mult)
    tt(msq[:], mv[:, 1:2], msq[:], ALU.add)
    ts_(ccs[:, 1:2], msq[:], float(HW), None, ALU.mult)
    cc_in = dram.tile([CHO, 2], F32)
    cc_out = dram.tile([CHO, 2], F32)
    nc.sync.dma_start(cc_in[:], ccs[:])
    nc.gpsimd.collective_compute(
        "AllReduce", ALU.add, replica_groups=[list(range(n_cores))],
        ins=[cc_in.opt()], outs=[cc_out.opt()])
    st = bn.tile([CHO, 2], F32)
    nc.sync.dma_start(st[:], cc_out[:])
    inv = 1.0 / float(n_cores * HW)
    mu = bn.tile([CHO, 1], F32); ts_(mu[:], st[:, 0:1], inv, None, ALU.mult)
    ex2 = bn.tile([CHO, 1], F32); ts_(ex2[:], st[:, 1:2], inv, None, ALU.mult)
    m2 = bn.tile([CHO, 1], F32); tt(m2[:], mu[:], mu[:], ALU.mult)
    var = bn.tile([CHO, 1], F32); tt(var[:], ex2[:], m2[:], ALU.subtract)
    epsb = bn.tile([CHO, 1], F32)
    nc.vector.memset(epsb[:], EPS)
    sd = bn.tile([CHO, 1], F32)
    nc.scalar.activation(sd[:], var[:], AF.Sqrt, bias=epsb[:])
    rsd = bn.tile([CHO, 1], F32)
    nc.vector.reciprocal(rsd[:], sd[:])
    sc = bn.tile([CHO, 1], F32); tt(sc[:], rsd[:], gam[:], ALU.mult)
    msc = bn.tile([CHO, 1], F32); tt(msc[:], mu[:], sc[:], ALU.mult)
    bb = bn.tile([CHO, 1], F32); tt(bb[:], bet[:], msc[:], ALU.subtract)
    out_sb = bn.tile([CHO, HW], F32)
    for c4 in range(4):
        sl = slice(1024 * c4, 1024 * c4 + 1024)
        nc.scalar.activation(out_sb[:, sl], out_ps[:, sl], AF.Relu,
                             bias=bb[:], scale=sc[:])
        nc.sync.dma_start(
            _ap(out_d.ap(), 1024 * c4, [[HW, CHO], [1, 1024]]),
            out_sb[:, sl])


# ---------------- host side ----------------------------------------------

_PERM = [2 * k for k in range(KK)] + [2 * k + 1 for k in range(KK)] + \
        [2 * KK + k for k in range(KK)]


def host_inputs(x, off_w, off_b, w, b, gamma, beta):
    """Per-core input maps (core i gets sample i)."""
    x = np.asarray(x, np.float32)
    off_w = np.asarray(off_w, np.float32)
    off_b = np.asarray(off_b, np.float32)
    w = np.asarray(w, np.float32)
    gamma = np.asarray(gamma, np.float32)
    beta = np.asarray(beta, np.float32)

    offw_r = off_w[_PERM]                                   # [27,128,3,3]
    offw_t = np.ascontiguousarray(
        offw_r.reshape(27, CHI, 9).transpose(2, 1, 0))      # [9,128,27]
    offb_r = off_b[_PERM]
    w_t = np.ascontiguousarray(
        w.reshape(CHO, CHI, 9).transpose(2, 1, 0)).astype(ml_dtypes.bfloat16)

    q = np.arange(4)[:, None, None]          # chunk
    k = np.arange(KK)[None, :, None]         # tap
    c = np.arange(1024)[None, None, :]       # col
    ymap = 4.0 * (c // 64) + q               # y of slot
    xmap = c % 64                            # x of slot
    gridy_h = np.ascontiguousarray(np.broadcast_to(
        ymap - 1.0 + k // 3 + offb_r[:KK][None, :, None],
        (4, KK, 1024))).reshape(36, 1024)
    gridx_h = np.ascontiguousarray(np.broadcast_to(
        xmap - 1.0 + k % 3 + offb_r[KK:2 * KK][None, :, None],
        (4, KK, 1024))).reshape(36, 1024)
    gridy = np.zeros((100, 1024), np.float32)
    gridy[0:36] = gridy_h
    gridy[64:100] = gridx_h
    offbm = np.tile(offb_r[2 * KK:], 4).reshape(36, 1)

    shared = {
        "offw": offw_t.astype(ml_dtypes.bfloat16),
        "w": w_t,
        "gridy": np.ascontiguousarray(gridy, np.float32),
        "offbm": np.ascontiguousarray(offbm, np.float32),
        "gamma": gamma, "beta": beta,
    }
    return [dict(shared, x=np.ascontiguousarray(x[i].reshape(CHI, HW)))
            for i in range(B)]


_NC_CACHE = {}


def _get_nc(n_cores=8):
    if n_cores not in _NC_CACHE:
        _NC_CACHE[n_cores] = build_kernel(n_cores)
    return _NC_CACHE[n_cores]


def kernel(x, off_w, off_b, w, b, gamma, beta):
    nc = _get_nc(8)
    in_maps = host_inputs(x, off_w, off_b, w, b, gamma, beta)
    res = None
    for attempt in range(3):
        try:
            res = run_bass_kernel_spmd(nc, in_maps, core_ids=list(range(8)))
            break
        except Exception:
            # a crashed prior session can leave a core in
            # NRT_EXEC_UNIT_UNRECOVERABLE; a fresh attempt resets it
            if attempt == 2:
                raise
    out = np.stack([res.results[i]["out"] for i in range(8)], axis=0)
    return out.reshape(B, CHO, H, W).astype(np.float32)
